# revision 1
# baseline (speedup 1.0000x reference)
"""Trainium2 Bass kernel for CSAttention.

Reference computation (per batch b of 32, N=1024 tokens, C=512 channels,
L=512 latent):
    qk  = x @ W_qk.T + b_qk            # [N, 2L]
    q   = qk[:, :L] * L**-0.5
    k   = qk[:, L:]
    out = softmax(q @ k.T, -1) @ y     # [N, C]

Sharding: data-parallel over the batch axis across 8 NeuronCores
(4 batches per core); W_qk / b_qk replicated.

Per-core kernel structure (bf16 matmul operands, fp32 PSUM accumulate):
  load:    x/y/W stream in via gpsimd casting DMA (f32 dram -> bf16 sbuf);
           x and W are transposed with PE transposes + DVE copy-back:
           XT [C, N], WT [C, 2L]
  stage B: QKT[l, n] = (WT col-slice).T @ XT          (+bias)   [2L, N]
  stage C: ST[m, n]  = (KT col-slice).T @ QT   -> exp(scale*.)  [N, N]
  stage D: out[n, :] = (ET col-slice).T @ [Y | 1]; the appended ones
           column produces the softmax denominator inside the same PSUM
           accumulation; normalize with DVE reciprocal + per-partition mul.
"""

import numpy as np

import concourse.bass as bass
import concourse.mybir as mybir
import concourse.tile as tile
from concourse import bacc
from concourse.bass_utils import run_bass_kernel_spmd
from concourse.masks import make_identity

P = 128
N_CORES = 8
B_FULL = 32
B_PER_CORE = B_FULL // N_CORES  # 4
N = 1024            # tokens
C = 512             # channels
L = 512             # latent
TWO_L = 2 * L
NT = N // P         # 8 token tiles
CT = C // P         # 4 channel tiles
LT = TWO_L // P     # 8 latent tiles (0..3 = q, 4..7 = k)
SCALE = float(L) ** -0.5
YA = C + 1          # augmented Y width: [Y | ones]
NA = 257            # first stage-D matmul free dim
NB = YA - NA        # 256
SCOL = C - NA       # ones column's index within psum_B (= 255)

F32 = mybir.dt.float32
BF16 = mybir.dt.bfloat16
IDENT = mybir.ActivationFunctionType.Identity
EXP = mybir.ActivationFunctionType.Exp


def _emit(tc, x, y, w, bvec, out):
    """Emit the per-core kernel. x/y: [B_PER_CORE, N, C] dram APs,
    w: [2L, C], bvec: [2L], out: [B_PER_CORE, N, C]."""
    from contextlib import ExitStack

    nc = tc.nc
    with ExitStack() as ctx:
        const = ctx.enter_context(tc.tile_pool(name="const", bufs=1))
        big = ctx.enter_context(tc.tile_pool(name="big", bufs=1))
        outp = ctx.enter_context(tc.tile_pool(name="outp", bufs=4))
        rsp = ctx.enter_context(tc.tile_pool(name="rsp", bufs=4))
        ps_mm = ctx.enter_context(tc.tile_pool(name="ps_mm", bufs=3, space="PSUM"))
        ps_d = ctx.enter_context(tc.tile_pool(name="ps_d", bufs=3, space="PSUM"))
        ps_tr = ctx.enter_context(tc.tile_pool(name="ps_tr", bufs=2, space="PSUM"))

        # b_qk striped so partition p, col t  <-  b_qk[t*128 + p]
        bias_sb = const.tile([P, LT], F32, tag="bias")
        nc.sync.dma_start(bias_sb, bvec.rearrange("(o p) -> p o", p=P))

        # ---- one-time: WT[c, l] = W[l, c], via cast-DMA + PE transpose ----
        wbf = big.tile([P, LT, C], BF16, tag="wbf")
        w_tiled = w.rearrange("(t p) c -> p t c", p=P)
        wt = big.tile([P, CT, TWO_L], BF16, tag="wt")

        identity = const.tile([P, P], BF16, tag="ident")
        make_identity(nc, identity)

        # ---- persistent per-batch workspaces ----
        xbf2 = [
            big.tile([P, NT, C], BF16, tag=f"xbf{j}", name=f"xbf{j}")
            for j in range(2)
        ]
        xt2 = [
            big.tile([P, CT, N], BF16, tag=f"xt{j}", name=f"xt{j}")
            for j in range(2)
        ]
        qkt = big.tile([P, LT, N], BF16, tag="qkt")     # rows l, cols n
        et = big.tile([P, NT, N], BF16, tag="et")       # rows m, cols n
        yaug = big.tile([P, NT, YA], BF16, tag="yaug")  # [Y | ones | pad]
        nc.vector.memset(yaug[:, :, C:YA], 1.0)

        def load_x(i):
            x_tiled = x[i].rearrange("(t p) c -> p t c", p=P)
            for j in range(2):
                nc.gpsimd.dma_start(
                    xbf2[i % 2][:, 4 * j:4 * j + 4], x_tiled[:, 4 * j:4 * j + 4]
                )

        def transpose_x(i):
            src, dst = xbf2[i % 2], xt2[i % 2]
            for nt_ in range(NT):
                for ct in range(CT):
                    ps = ps_tr.tile([P, P], BF16, tag="tr")
                    nc.tensor.transpose(
                        ps, src[:, nt_, ct * P:(ct + 1) * P], identity
                    )
                    nc.vector.tensor_copy(dst[:, ct, nt_ * P:(nt_ + 1) * P], ps)

        def load_y(i):
            nc.gpsimd.dma_start(
                yaug[:, :, 0:C], y[i].rearrange("(t p) c -> p t c", p=P)
            )

        x0_tiled = x[0].rearrange("(t p) c -> p t c", p=P)
        nc.gpsimd.dma_start(xbf2[0][:, 0:4], x0_tiled[:, 0:4])
        for a, b in ((0, 1), (1, 2), (2, 4), (4, 8)):
            nc.gpsimd.dma_start(wbf[:, a:b], w_tiled[:, a:b])
        nc.gpsimd.dma_start(xbf2[0][:, 4:8], x0_tiled[:, 4:8])
        # x nh0-half transposes first (B's rhs), then W (B's lhsT), then x nh1
        for nt_ in range(4):
            for ct in range(CT):
                ps = ps_tr.tile([P, P], BF16, tag="tr")
                nc.tensor.transpose(ps, xbf2[0][:, nt_, ct * P:(ct + 1) * P], identity)
                nc.vector.tensor_copy(xt2[0][:, ct, nt_ * P:(nt_ + 1) * P], ps)
        for lt in range(LT):
            for ct in range(CT):
                ps = ps_tr.tile([P, P], BF16, tag="tr")
                nc.tensor.transpose(
                    ps, wbf[:, lt, ct * P:(ct + 1) * P], identity
                )
                nc.vector.tensor_copy(wt[:, ct, lt * P:(lt + 1) * P], ps)
        for nt_ in range(4, NT):
            for ct in range(CT):
                ps = ps_tr.tile([P, P], BF16, tag="tr")
                nc.tensor.transpose(ps, xbf2[0][:, nt_, ct * P:(ct + 1) * P], identity)
                nc.vector.tensor_copy(xt2[0][:, ct, nt_ * P:(nt_ + 1) * P], ps)

        for i in range(B_PER_CORE):
            if i + 1 < B_PER_CORE:
                load_x(i + 1)
            load_y(i)
            xt = xt2[i % 2]

            # ---- stage B: QKT = WT.T @ XT (+ bias) ----
            for nh in range(2):
                for lt in range(LT):
                    ps = ps_mm.tile([P, 512], F32, tag="mm")
                    for ct in range(CT):
                        nc.tensor.matmul(
                            ps,
                            wt[:, ct, lt * P:(lt + 1) * P],
                            xt[:, ct, nh * 512:(nh + 1) * 512],
                            start=(ct == 0),
                            stop=(ct == CT - 1),
                        )
                    nc.scalar.activation(
                        qkt[:, lt, nh * 512:(nh + 1) * 512],
                        ps,
                        IDENT,
                        bias=bias_sb[:, lt:lt + 1],
                    )

            # ---- stage C: ST[m, n] = K[m] . Q[n] ; ET = exp(scale * ST) ----
            for nh in range(2):
                for mt in range(NT):
                    ps = ps_mm.tile([P, 512], F32, tag="mm")
                    for lq in range(4):
                        nc.tensor.matmul(
                            ps,
                            qkt[:, 4 + lq, mt * P:(mt + 1) * P],
                            qkt[:, lq, nh * 512:(nh + 1) * 512],
                            start=(lq == 0),
                            stop=(lq == 3),
                        )
                    nc.scalar.activation(
                        et[:, mt, nh * 512:(nh + 1) * 512], ps, EXP, scale=SCALE
                    )

            # transpose next batch's x on the PE (between C and D)
            if i + 1 < B_PER_CORE:
                transpose_x(i + 1)

            # ---- stage D: out = ET.T @ [Y | 1], then normalize ----
            for nt_ in range(NT):
                psA = ps_d.tile([P, NA], F32, tag="d")
                psB = ps_d.tile([P, NA], F32, tag="d")
                for mt in range(NT):
                    lw = et[:, mt, nt_ * P:(nt_ + 1) * P]
                    nc.tensor.matmul(
                        psA, lw, yaug[:, mt, 0:NA],
                        start=(mt == 0), stop=(mt == NT - 1),
                    )
                    nc.tensor.matmul(
                        psB[:, 0:NB], lw, yaug[:, mt, NA:YA],
                        start=(mt == 0), stop=(mt == NT - 1),
                    )
                rs = rsp.tile([P, 1], F32, tag="rs")
                nc.vector.reciprocal(rs, psB[:, SCOL:SCOL + 1])
                ob = outp.tile([P, C], F32, tag="ob")
                nc.scalar.mul(ob[:, 0:NA], psA[:, 0:NA], rs)
                nc.vector.tensor_scalar_mul(ob[:, NA:C], psB[:, 0:SCOL], rs)
                nc.sync.dma_start(out[i, nt_ * P:(nt_ + 1) * P, :], ob)


_NC_CACHE = {}


def _build():
    if "nc" in _NC_CACHE:
        return _NC_CACHE["nc"]
    nc = bacc.Bacc(
        "TRN2",
        target_bir_lowering=False,
        debug=False,
        enable_asserts=False,
        num_devices=N_CORES,
    )
    x = nc.dram_tensor("x", [B_PER_CORE, N, C], F32, kind="ExternalInput").ap()
    y = nc.dram_tensor("y", [B_PER_CORE, N, C], F32, kind="ExternalInput").ap()
    w = nc.dram_tensor("W_qk", [TWO_L, C], F32, kind="ExternalInput").ap()
    bvec = nc.dram_tensor("b_qk", [TWO_L], F32, kind="ExternalInput").ap()
    out = nc.dram_tensor("out", [B_PER_CORE, N, C], F32, kind="ExternalOutput").ap()
    with tile.TileContext(nc) as tc:
        _emit(tc, x, y, w, bvec, out)
    nc.compile()
    _NC_CACHE["nc"] = nc
    return nc


def run(x, y, W_qk, b_qk, trace=False):
    """Run the SPMD kernel on 8 cores; returns (out, BassKernelResults)."""
    nc = _build()
    x = np.ascontiguousarray(x, dtype=np.float32)
    y = np.ascontiguousarray(y, dtype=np.float32)
    W_qk = np.ascontiguousarray(W_qk, dtype=np.float32)
    b_qk = np.ascontiguousarray(b_qk, dtype=np.float32)
    in_maps = [
        {
            "x": x[k * B_PER_CORE:(k + 1) * B_PER_CORE],
            "y": y[k * B_PER_CORE:(k + 1) * B_PER_CORE],
            "W_qk": W_qk,
            "b_qk": b_qk,
        }
        for k in range(N_CORES)
    ]
    res = run_bass_kernel_spmd(
        nc, in_maps, core_ids=list(range(N_CORES)), trace=trace
    )
    outs = [r["out"] for r in res.results]
    return np.concatenate(outs, axis=0), res


def kernel(x, y, W_qk, b_qk):
    out, _ = run(x, y, W_qk, b_qk)
    return out

